# revision 19
# baseline (speedup 1.0000x reference)
"""Additive (Bahdanau) attention on 8 Trainium2 NeuronCores.

Problem (per reference):
    qp = queries @ W1q.T            [b=4, nq=64, h=512]
    kp = keys @ W1k.T + b1          [b=4, s=1024, h=512]
    scores[b,q,s] = sum_h w2[h] * tanh(qp[b,q,h] + kp[b,s,h])
    out = softmax_s(scores) @ values

Key trick: ACT (ScalarE) is the only tanh engine and evaluating tanh on the
full b*nq*s*h grid (134M elements) is ACT-bound at ~150us/core.  Instead we
expand tanh in an odd harmonic sine series on [-R, R]:

    tanh(x) ~= sum_k b_k sin(k*w1*x),   w1 = pi/L

and use sin(a+b) = sin a cos b + cos a sin b, which FACTORIZES each term over
the h axis into a matmul:

    scores = sum_k [ (w2*b_k*sin_k(qp)) @ cos_k(kp).T
                   + (w2*b_k*cos_k(qp)) @ sin_k(kp).T ]

so the bulk of the work becomes TensorE matmuls with contraction 512*2K.
The harmonics sin_k/cos_k are generated from the k=1 pair (ACT Sin, whose
valid input domain is only |x| <~ pi) via Chebyshev doubling/step recurrences
in fp16: the big keys-side chains on the VectorEngine, the small query-side
chains on GPSIMD so they run concurrently.

Sharding: 8 cores = (batch 4) x (key-sequence halves 2).  Each core computes
unnormalized out_u = exp(scores_local) @ values_local and den = sum_s
exp(scores_local); the host combines flash-style:
    final = (out_u0 + out_u1) / (den0 + den1).

Measured on HW (8 axon-tunneled NeuronCores): final max-abs relative
error ~7e-4 vs the fp32 reference at K_HARM=6 (~1.7e-4 at K_HARM=8).
"""

import math

import numpy as np

import concourse.bacc as bacc
import concourse.mybir as mybir
import concourse.tile as tile
from concourse.bass_utils import run_bass_kernel_spmd

AF = mybir.ActivationFunctionType
ALU = mybir.AluOpType
F32 = mybir.dt.float32
F16 = mybir.dt.float16

# ---- problem shape (hardcoded; kernel.py must be self-contained) ----
B, NQ, S, D, H, V = 4, 64, 1024, 512, 512, 512
NCORES = 8
SL = S // 2          # s per core (batch x s-half sharding)

# ---- tanh ~= sum_k BCOEF[k] sin((k+1)*W1*x) fit on [-R, R] ----
K_HARM = 6
L_PERIOD = 6.0
OMEGA1 = math.pi / L_PERIOD
R_FIT = 4.0


def _fit_coeffs() -> np.ndarray:
    xs = np.linspace(-R_FIT, R_FIT, 20001)
    A = np.sin(np.outer(xs, np.arange(1, K_HARM + 1) * OMEGA1))
    b, *_ = np.linalg.lstsq(A, np.tanh(xs), rcond=None)
    return b.astype(np.float64)


_BCOEF = _fit_coeffs()

_PROG_CACHE: dict = {}


def _chain_step(nc, eng, pool, Sd, Cd, two_c1, ones, k, width, tag):
    """One Chebyshev recurrence step: S_k, C_k from lower harmonics on `eng`.

    S_k = 2C1*S_{k-1} - S_{k-2} (S0=0 so S2 is a bare product); even S_k on
    the DVE uses the fused doubling (2*S_m)*C_m (scalar_tensor_tensor is not
    a GPSIMD opcode).  C_k = 2C1*C_{k-1} - C_{k-2} with C0 = 1 (the DVE gets
    -1 via a 4x-rate immediate tensor_scalar; GPSIMD subtracts a ones tile).
    """
    Sd[k] = pool.tile([128, width], F16, tag=f"{tag}S{k}", name=f"{tag}S{k}")
    Cd[k] = pool.tile([128, width], F16, tag=f"{tag}C{k}", name=f"{tag}C{k}")
    if k == 2:
        eng.tensor_mul(Sd[2][:], two_c1[:], Sd[1][:])
    elif k % 2 == 0 and eng is nc.vector:
        m = k // 2
        eng.scalar_tensor_tensor(
            Sd[k][:], Sd[m][:], 2.0, Cd[m][:], ALU.mult, ALU.mult)
    else:
        eng.tensor_mul(Sd[k][:], two_c1[:], Sd[k - 1][:])
        eng.tensor_sub(Sd[k][:], Sd[k][:], Sd[k - 2][:])
    eng.tensor_mul(Cd[k][:], two_c1[:], Cd[k - 1][:])
    if k > 2:
        eng.tensor_sub(Cd[k][:], Cd[k][:], Cd[k - 2][:])
    elif ones is not None:
        eng.tensor_sub(Cd[k][:], Cd[k][:], ones[:])
    else:
        eng.tensor_scalar_add(Cd[k][:], Cd[k][:], -1.0)


def _build_program():
    nc = bacc.Bacc("TRN2", target_bir_lowering=False, debug=False,
                   num_devices=NCORES)

    d_qT = nc.dram_tensor("qT", [128, 4 * NQ], F16, kind="ExternalInput").ap()
    d_keysT = nc.dram_tensor("keysT", [128, 4 * SL], F16, kind="ExternalInput").ap()
    d_vals = nc.dram_tensor("vals", [128, 4 * V], F16, kind="ExternalInput").ap()
    d_w1qT = nc.dram_tensor("w1qT", [128, 16 * 128], F16, kind="ExternalInput").ap()
    d_w1kT = nc.dram_tensor("w1kT", [128, 16 * 128], F16, kind="ExternalInput").ap()
    d_b1r = nc.dram_tensor("b1r", [1, 512], F16, kind="ExternalInput").ap()
    d_onesr = nc.dram_tensor("ones_r", [1, 512], F16, kind="ExternalInput").ap()
    d_w2b = nc.dram_tensor("w2b", [128, K_HARM * 4 * NQ], F16,
                           kind="ExternalInput").ap()
    d_eye = nc.dram_tensor("eye64", [NQ, NQ], F16, kind="ExternalInput").ap()
    d_outu = nc.dram_tensor("out_u", [NQ, V], F32, kind="ExternalOutput").ap()
    d_den = nc.dram_tensor("den", [NQ, 1], F32, kind="ExternalOutput").ap()

    with tile.TileContext(nc) as tc:
        with tc.tile_pool(name="c", bufs=1) as cp, \
             tc.tile_pool(name="pp", bufs=1, space="PSUM") as pp, \
             tc.tile_pool(name="sp", bufs=1, space="PSUM") as sp, \
             tc.tile_pool(name="tp", bufs=2, space="PSUM") as tp:

            # ---- input loads (chunked, critical kp inputs first) ----
            keysT = cp.tile([128, 4 * SL], F16, tag="keysT", name="keysT")
            w1kT = cp.tile([128, 16 * 128], F16, tag="w1kT", name="w1kT")
            nc.sync.dma_start(w1kT[:, 0:512], d_w1kT[:, 0:512])
            for dc in range(4):
                nc.sync.dma_start(keysT[:, dc * SL:(dc + 1) * SL],
                                  d_keysT[:, dc * SL:(dc + 1) * SL])
            for hc in range(1, 4):
                nc.sync.dma_start(w1kT[:, hc * 512:(hc + 1) * 512],
                                  d_w1kT[:, hc * 512:(hc + 1) * 512])
            b1r = cp.tile([1, 512], F16, tag="b1r", name="b1r")
            nc.sync.dma_start(b1r[:], d_b1r[:])
            onesr = cp.tile([1, 512], F16, tag="onesr", name="onesr")
            nc.sync.dma_start(onesr[:], d_onesr[:])
            qT = cp.tile([128, 4 * NQ], F16, tag="qT", name="qT")
            nc.sync.dma_start(qT[:], d_qT[:])
            w1qT = cp.tile([128, 16 * 128], F16, tag="w1qT", name="w1qT")
            nc.sync.dma_start(w1qT[:], d_w1qT[:])
            w2b = cp.tile([128, K_HARM * 4 * NQ], F16, tag="w2b", name="w2b")
            nc.sync.dma_start(w2b[:], d_w2b[:])
            eye = cp.tile([NQ, NQ], F16, tag="eye", name="eye")
            nc.sync.dma_start(eye[:], d_eye[:])
            vals = cp.tile([128, 4 * V], F16, tag="vals", name="vals")
            for dc in range(4):
                nc.sync.dma_start(vals[:, dc * V:(dc + 1) * V],
                                  d_vals[:, dc * V:(dc + 1) * V])

            halfpi = cp.tile([128, 1], F32, tag="halfpi", name="halfpi")
            nc.vector.memset(halfpi[:], float(np.pi / 2))
            # dummy tiny Sin up front so the ~2.7us ACT table load overlaps
            # the DMA/projection phase instead of gating the base sins
            warm = cp.tile([128, 1], F16, tag="warm", name="warm")
            nc.scalar.activation(warm[:], halfpi[:], AF.Sin)

            # ---- projections (fp16 inputs: 1 PE cycle/row vs 4 for fp32).
            # Both stay in PSUM; the base sin/cos ACTs read PSUM directly.
            # b1 joins kp inside the PSUM accumulation as a rank-1 matmul
            # term (b1 row x ones row), so one full-width ACT per base
            # function suffices. ----
            kps = pp.tile([128, 4 * SL], F32, tag="pk", name="kps")
            for hc in range(4):
                for dc in range(4):
                    nc.tensor.matmul(
                        kps[:, hc * SL:(hc + 1) * SL],
                        w1kT[:, (hc * 4 + dc) * 128:(hc * 4 + dc + 1) * 128],
                        keysT[:, dc * SL:(dc + 1) * SL],
                        start=(dc == 0), stop=False)
                # b1 enters as a rank-1 term: kps[h, s] += b1[h] * 1
                nc.tensor.matmul(
                    kps[:, hc * SL:(hc + 1) * SL],
                    b1r[:, hc * 128:(hc + 1) * 128], onesr[:],
                    start=False, stop=True)
            qps = sp.tile([128, 4 * NQ], F32, tag="sc", name="qps")
            for hc in range(4):
                for dc in range(4):
                    nc.tensor.matmul(
                        qps[:, hc * NQ:(hc + 1) * NQ],
                        w1qT[:, (hc * 4 + dc) * 128:(hc * 4 + dc + 1) * 128],
                        qT[:, dc * NQ:(dc + 1) * NQ],
                        start=(dc == 0), stop=(dc == 3))

            # ---- harmonic bases + score matmuls, interleaved per k so
            # every engine (ACT base sins -> DVE keys-chains + w2b mults,
            # GPSIMD query-chains, PE matmuls) pipelines with the others ----
            SK, CK, SQ, CQ = {}, {}, {}, {}
            SK[1] = cp.tile([128, 4 * SL], F16, tag="kS1", name="kS1")
            CK[1] = cp.tile([128, 4 * SL], F16, tag="kC1", name="kC1")
            nc.scalar.activation(SK[1][:], kps[:], AF.Sin, scale=OMEGA1)
            nc.scalar.activation(CK[1][:], kps[:], AF.Sin,
                                 bias=halfpi[:, 0:1], scale=OMEGA1)
            SQ[1] = cp.tile([128, 4 * NQ], F16, tag="qS1", name="qS1")
            nc.scalar.activation(SQ[1][:], qps[:], AF.Sin, scale=OMEGA1)
            CQ[1] = cp.tile([128, 4 * NQ], F16, tag="qC1", name="qC1")
            nc.scalar.activation(CQ[1][:], qps[:], AF.Sin,
                                 bias=halfpi[:, 0:1], scale=OMEGA1)
            two_c1k = cp.tile([128, 4 * SL], F16, tag="k2C1", name="k2C1")
            nc.vector.tensor_scalar_mul(two_c1k[:], CK[1][:], 2.0)
            two_c1q = cp.tile([128, 4 * NQ], F16, tag="q2C1", name="q2C1")
            nc.gpsimd.tensor_add(two_c1q[:], CQ[1][:], CQ[1][:])
            ones_q = cp.tile([128, 4 * NQ], F16, tag="qones", name="qones")
            nc.gpsimd.memset(ones_q[:], 1.0)

            # scores = sum_k (w2 b_k sin_k(qp))^T cos_k(kp)
            #        + (w2 b_k cos_k(qp))^T sin_k(kp)  in one PSUM bank
            sc = sp.tile([NQ, SL], F32, tag="sc", name="sc")
            n_mm = K_HARM * 2 * 4
            i_mm = 0
            for k in range(1, K_HARM + 1):
                if k >= 2:
                    _chain_step(nc, nc.vector, cp, SK, CK, two_c1k, None,
                                k, 4 * SL, "k")
                    _chain_step(nc, nc.gpsimd, cp, SQ, CQ, two_c1q, ones_q,
                                k, 4 * NQ, "q")
                sqw = cp.tile([128, 4 * NQ], F16, tag=f"sqw{k}", name=f"sqw{k}")
                nc.gpsimd.tensor_mul(
                    sqw[:], SQ[k][:], w2b[:, (k - 1) * 4 * NQ:k * 4 * NQ])
                cqw = cp.tile([128, 4 * NQ], F16, tag=f"cqw{k}", name=f"cqw{k}")
                nc.gpsimd.tensor_mul(
                    cqw[:], CQ[k][:], w2b[:, (k - 1) * 4 * NQ:k * 4 * NQ])
                for qside, kside in ((cqw, SK[k]), (sqw, CK[k])):
                    for hc in range(4):
                        nc.tensor.matmul(
                            sc[:],
                            qside[:, hc * NQ:(hc + 1) * NQ],
                            kside[:, hc * SL:(hc + 1) * SL],
                            start=(i_mm == 0), stop=(i_mm == n_mm - 1))
                        i_mm += 1

            # ---- softmax numerator/denominator (no max-subtract needed:
            #      |scores| <= sum|w2| ~ 11.5, exp stays in fp32 range) ----
            esc = cp.tile([NQ, SL], F16, tag="esc", name="esc")
            den = cp.tile([NQ, 1], F32, tag="den", name="den")
            nc.scalar.activation(esc[:], sc[:], AF.Exp, accum_out=den[:])

            # ---- out_u = exp(scores) @ values  (transpose exp via PE) ----
            av = sp.tile([NQ, V], F32, tag="av", name="av")
            for i in range(4):
                pt = tp.tile([128, NQ], F16, tag="pt", name="pt")
                nc.tensor.transpose(pt[:], esc[:, i * 128:(i + 1) * 128], eye[:])
                et = cp.tile([128, NQ], F16, tag=f"et{i}", name=f"et{i}")
                nc.vector.tensor_copy(et[:], pt[:])
                nc.tensor.matmul(av[:], et[:], vals[:, i * V:(i + 1) * V],
                                 start=(i == 0), stop=(i == 3))
            outu = cp.tile([NQ, V], F32, tag="outu", name="outu")
            nc.scalar.copy(outu[:], av[:])
            nc.sync.dma_start(d_outu[:], outu[:])
            nc.sync.dma_start(d_den[:], den[:])

    nc.compile()
    return nc


def _prep_inputs(queries, keys, values, W1, b1, w2):
    """Per-core input dicts.  Core c = batch (c//2), s-half (c%2)."""
    W1q = np.ascontiguousarray(W1[:, :D])
    W1k = np.ascontiguousarray(W1[:, D:])
    # lhsT blocks [d, h], hc-major: free index = (hc*4+dc)*128 + h_in so the
    # projection for output chunk hc only needs one contiguous DMA chunk
    w1qT = W1q.T.reshape(4, 128, 4, 128).transpose(1, 2, 0, 3).reshape(128, -1)
    w1kT = W1k.T.reshape(4, 128, 4, 128).transpose(1, 2, 0, 3).reshape(128, -1)
    b1r = b1.reshape(1, 512).astype(np.float16)
    ones_r = np.ones((1, 512), dtype=np.float16)
    # w2b[p, ((k*4)+hc)*NQ + q] = w2[hc*128+p] * BCOEF[k]  (replicated over q)
    w2hc = w2.reshape(4, 128).T                          # [128, 4]
    w2b = (w2hc[:, None, :, None] * _BCOEF[None, :, None, None]
           ).astype(np.float16).reshape(128, K_HARM * 4, 1)
    w2b = np.broadcast_to(w2b, (128, K_HARM * 4, NQ)).reshape(128, -1)
    w2b = np.ascontiguousarray(w2b)
    eye = np.eye(NQ, dtype=np.float16)

    in_maps = []
    for c in range(NCORES):
        bi, sh = c // 2, c % 2
        qT = queries[bi].T.reshape(4, 128, NQ).transpose(1, 0, 2).reshape(128, -1)
        keysT = (keys[bi, sh * SL:(sh + 1) * SL].T
                 .reshape(4, 128, SL).transpose(1, 0, 2).reshape(128, -1))
        vals = (values[bi, sh * SL:(sh + 1) * SL]
                .reshape(4, 128, V).transpose(1, 0, 2).reshape(128, -1))
        in_maps.append({
            "qT": np.ascontiguousarray(qT, dtype=np.float16),
            "keysT": np.ascontiguousarray(keysT, dtype=np.float16),
            "vals": np.ascontiguousarray(vals, dtype=np.float16),
            "w1qT": np.ascontiguousarray(w1qT, dtype=np.float16),
            "w1kT": np.ascontiguousarray(w1kT, dtype=np.float16),
            "b1r": b1r,
            "ones_r": ones_r,
            "w2b": w2b,
            "eye64": eye,
        })
    return in_maps


def run(inputs_by_core, trace=False, **kw):
    if "nc" not in _PROG_CACHE:
        _PROG_CACHE["nc"] = _build_program()
    return run_bass_kernel_spmd(_PROG_CACHE["nc"], inputs_by_core,
                                core_ids=list(range(NCORES)), trace=trace, **kw)


def kernel(queries, keys, values, W1, b1, w2):
    queries = np.asarray(queries, dtype=np.float32)
    keys = np.asarray(keys, dtype=np.float32)
    values = np.asarray(values, dtype=np.float32)
    W1 = np.asarray(W1, dtype=np.float32)
    b1 = np.asarray(b1, dtype=np.float32)
    w2 = np.asarray(w2, dtype=np.float32)

    res = run(_prep_inputs(queries, keys, values, W1, b1, w2))
    out = np.empty((B, NQ, V), dtype=np.float32)
    for bi in range(B):
        u0, d0 = res.results[2 * bi]["out_u"], res.results[2 * bi]["den"]
        u1, d1 = res.results[2 * bi + 1]["out_u"], res.results[2 * bi + 1]["den"]
        out[bi] = (u0 + u1) / (d0 + d1)
    return out



# revision 29
# speedup vs baseline: 1.1105x; 1.1105x over previous
"""Additive (Bahdanau) attention on 8 Trainium2 NeuronCores.

Problem (per reference):
    qp = queries @ W1q.T            [b=4, nq=64, h=512]
    kp = keys @ W1k.T + b1          [b=4, s=1024, h=512]
    scores[b,q,s] = sum_h w2[h] * tanh(qp[b,q,h] + kp[b,s,h])
    out = softmax_s(scores) @ values

Key trick: ACT (ScalarE) is the only tanh engine and evaluating tanh on the
full b*nq*s*h grid (134M elements) is ACT-bound at ~150us/core.  Instead we
expand tanh in an odd harmonic sine series on [-R, R]:

    tanh(x) ~= sum_k b_k sin(k*w1*x),   w1 = pi/L

and use sin(a+b) = sin a cos b + cos a sin b, which FACTORIZES each term over
the h axis into a matmul:

    scores = sum_k [ (w2*b_k*sin_k(qp)) @ cos_k(kp).T
                   + (w2*b_k*cos_k(qp)) @ sin_k(kp).T ]

so the bulk of the work becomes TensorE matmuls with contraction 512*2K.
The harmonics sin_k/cos_k are generated from the k=1 pair (ACT Sin, whose
valid input domain is only |x| <~ pi) via Chebyshev doubling/step recurrences
in fp16: the big keys-side chains on the VectorEngine, the small query-side
chains on GPSIMD so they run concurrently.

Sharding: 8 cores = (batch 4) x (key-sequence halves 2).  Each core computes
unnormalized out_u = exp(scores_local) @ values_local and den = sum_s
exp(scores_local); the host combines flash-style:
    final = (out_u0 + out_u1) / (den0 + den1).

Measured on HW (8 axon-tunneled NeuronCores): final max-abs relative
error ~7e-4 vs the fp32 reference at K_HARM=6 (~1.7e-4 at K_HARM=8).
"""

import math

import numpy as np

import concourse.bacc as bacc
import concourse.mybir as mybir
import concourse.tile as tile
from concourse.bass_utils import run_bass_kernel_spmd

AF = mybir.ActivationFunctionType
ALU = mybir.AluOpType
F32 = mybir.dt.float32
F16 = mybir.dt.float16

# ---- problem shape (hardcoded; kernel.py must be self-contained) ----
B, NQ, S, D, H, V = 4, 64, 1024, 512, 512, 512
NCORES = 8
SL = S // 2          # s per core (batch x s-half sharding)

# ---- tanh ~= sum_k BCOEF[k] sin((k+1)*W1*x) fit on [-R, R] ----
K_HARM = 6
L_PERIOD = 6.0
OMEGA1 = math.pi / L_PERIOD
R_FIT = 4.0


def _fit_coeffs() -> np.ndarray:
    xs = np.linspace(-R_FIT, R_FIT, 20001)
    A = np.sin(np.outer(xs, np.arange(1, K_HARM + 1) * OMEGA1))
    b, *_ = np.linalg.lstsq(A, np.tanh(xs), rcond=None)
    return b.astype(np.float64)


_BCOEF = _fit_coeffs()

_PROG_CACHE: dict = {}


def SIGMA_BASE(m):
    """sigma_m for the stored-scale convention: odd m -> 1, even m -> 2*sigma_{m/2}."""
    return 1 if m % 2 else 2 * SIGMA_BASE(m // 2)


def _chain_step(nc, eng, pool, Sd, Cd, two_c1, ones, k, width, tag):
    """One Chebyshev recurrence step: S_k, C_k from lower harmonics on `eng`.

    S_k = 2C1*S_{k-1} - S_{k-2} (S0=0 so S2 is a bare product); even S_k on
    the DVE uses the fused doubling (2*S_m)*C_m (scalar_tensor_tensor is not
    a GPSIMD opcode).  C_k = 2C1*C_{k-1} - C_{k-2} with C0 = 1 (the DVE gets
    -1 via a 4x-rate immediate tensor_scalar; GPSIMD subtracts a ones tile).
    """
    Sd[k] = pool.tile([128, width], F16, tag=f"{tag}S{k}", name=f"{tag}S{k}")
    Cd[k] = pool.tile([128, width], F16, tag=f"{tag}C{k}", name=f"{tag}C{k}")
    if k == 2:
        eng.tensor_mul(Sd[2][:], two_c1[:], Sd[1][:])
    elif k % 2 == 0 and eng is nc.vector:
        m = k // 2
        eng.scalar_tensor_tensor(
            Sd[k][:], Sd[m][:], 2.0, Cd[m][:], ALU.mult, ALU.mult)
    else:
        eng.tensor_mul(Sd[k][:], two_c1[:], Sd[k - 1][:])
        eng.tensor_sub(Sd[k][:], Sd[k][:], Sd[k - 2][:])
    eng.tensor_mul(Cd[k][:], two_c1[:], Cd[k - 1][:])
    if k > 2:
        eng.tensor_sub(Cd[k][:], Cd[k][:], Cd[k - 2][:])
    elif ones is not None:
        eng.tensor_sub(Cd[k][:], Cd[k][:], ones[:])
    else:
        eng.tensor_scalar_add(Cd[k][:], Cd[k][:], -1.0)


def _build_program():
    nc = bacc.Bacc("TRN2", target_bir_lowering=False, debug=False,
                   num_devices=NCORES)

    d_qT = nc.dram_tensor("qT", [128, 4 * NQ], F16, kind="ExternalInput").ap()
    d_keysT = nc.dram_tensor("keysT", [128, 4 * SL], F16, kind="ExternalInput").ap()
    d_vals = nc.dram_tensor("vals", [128, 4 * V], F16, kind="ExternalInput").ap()
    d_w1qT = nc.dram_tensor("w1qT", [128, 16 * 128], F16, kind="ExternalInput").ap()
    d_w1kT = nc.dram_tensor("w1kT", [128, 16 * 128], F16, kind="ExternalInput").ap()
    d_b1r = nc.dram_tensor("b1r", [1, 512], F16, kind="ExternalInput").ap()
    d_onesr = nc.dram_tensor("ones_r", [1, 512], F16, kind="ExternalInput").ap()
    d_w2b = nc.dram_tensor("w2b", [128, K_HARM * 4 * NQ], F16,
                           kind="ExternalInput").ap()
    d_w2bc = nc.dram_tensor("w2bc", [128, K_HARM * 4 * NQ], F16,
                            kind="ExternalInput").ap()
    d_eye = nc.dram_tensor("eye64", [NQ, NQ], F16, kind="ExternalInput").ap()
    d_outu = nc.dram_tensor("out_u", [NQ, V], F32, kind="ExternalOutput").ap()
    d_den = nc.dram_tensor("den", [NQ, 1], F32, kind="ExternalOutput").ap()

    with tile.TileContext(nc) as tc:
        with tc.tile_pool(name="c", bufs=1) as cp, \
             tc.tile_pool(name="pp", bufs=1, space="PSUM") as pp, \
             tc.tile_pool(name="sp", bufs=1, space="PSUM") as sp, \
             tc.tile_pool(name="tp", bufs=2, space="PSUM") as tp:

            # ---- input loads (chunked, critical kp inputs first) ----
            b1r = cp.tile([1, 512], F16, tag="b1r", name="b1r")
            nc.sync.dma_start(b1r[:], d_b1r[:])
            onesr = cp.tile([1, 512], F16, tag="onesr", name="onesr")
            nc.sync.dma_start(onesr[:], d_onesr[:])
            keysT = cp.tile([128, 4 * SL], F16, tag="keysT", name="keysT")
            w1kT = cp.tile([128, 16 * 128], F16, tag="w1kT", name="w1kT")
            nc.sync.dma_start(w1kT[:, 0:512], d_w1kT[:, 0:512])
            for dc in range(4):
                nc.sync.dma_start(keysT[:, dc * SL:(dc + 1) * SL],
                                  d_keysT[:, dc * SL:(dc + 1) * SL])
            for hc in range(1, 4):
                nc.sync.dma_start(w1kT[:, hc * 512:(hc + 1) * 512],
                                  d_w1kT[:, hc * 512:(hc + 1) * 512])
            qT = cp.tile([128, 4 * NQ], F16, tag="qT", name="qT")
            nc.sync.dma_start(qT[:], d_qT[:])
            w1qT = cp.tile([128, 16 * 128], F16, tag="w1qT", name="w1qT")
            nc.sync.dma_start(w1qT[:], d_w1qT[:])
            w2b = cp.tile([128, K_HARM * 4 * NQ], F16, tag="w2b", name="w2b")
            nc.sync.dma_start(w2b[:], d_w2b[:])
            w2bc = cp.tile([128, K_HARM * 4 * NQ], F16, tag="w2bc", name="w2bc")
            nc.sync.dma_start(w2bc[:], d_w2bc[:])
            eye = cp.tile([NQ, NQ], F16, tag="eye", name="eye")
            nc.sync.dma_start(eye[:], d_eye[:])
            vals = cp.tile([128, 4 * V], F16, tag="vals", name="vals")
            for dc in range(4):
                nc.sync.dma_start(vals[:, dc * V:(dc + 1) * V],
                                  d_vals[:, dc * V:(dc + 1) * V])

            halfpi = cp.tile([128, 1], F32, tag="halfpi", name="halfpi")
            nc.vector.memset(halfpi[:], float(np.pi / 2))
            # pre-warm the PE during the DMA phase: the HAM clock gate only
            # releases full rate after ~3.4us of sustained activity, so a
            # burst of zero-matmuls lets the real projections run at 2.4GHz
            zsrc = cp.tile([128, 512], F16, tag="zsrc", name="zsrc")
            nc.vector.memset(zsrc[:], 0.0)
            zp = sp.tile([NQ, 512], F32, tag="sc", name="zp")
            for i in range(8):
                nc.tensor.matmul(zp[:], zsrc[:, :NQ], zsrc[:],
                                 start=(i == 0), stop=(i == 7))
            # dummy tiny Sin up front so the ~2.7us ACT table load overlaps
            # the DMA/projection phase instead of gating the base sins
            warm = cp.tile([128, 1], F16, tag="warm", name="warm")
            nc.scalar.activation(warm[:], halfpi[:], AF.Sin)

            # ---- projections (fp16 inputs: 1 PE cycle/row vs 4 for fp32).
            # Both stay in PSUM; the base sin/cos ACTs read PSUM directly.
            # b1 joins kp inside the PSUM accumulation as a rank-1 matmul
            # term (b1 row x ones row), so one full-width ACT per base
            # function suffices. ----
            kps = pp.tile([128, 4 * SL], F32, tag="pk", name="kps")
            for hc in range(4):
                for dc in range(4):
                    nc.tensor.matmul(
                        kps[:, hc * SL:(hc + 1) * SL],
                        w1kT[:, (hc * 4 + dc) * 128:(hc * 4 + dc + 1) * 128],
                        keysT[:, dc * SL:(dc + 1) * SL],
                        start=(dc == 0), stop=False)
                # b1 enters as a rank-1 term: kps[h, s] += b1[h] * 1
                nc.tensor.matmul(
                    kps[:, hc * SL:(hc + 1) * SL],
                    b1r[:, hc * 128:(hc + 1) * 128], onesr[:],
                    start=False, stop=True)
            qps = sp.tile([128, 4 * NQ], F32, tag="sc", name="qps")
            for hc in range(4):
                for dc in range(4):
                    nc.tensor.matmul(
                        qps[:, hc * NQ:(hc + 1) * NQ],
                        w1qT[:, (hc * 4 + dc) * 128:(hc * 4 + dc + 1) * 128],
                        qT[:, dc * NQ:(dc + 1) * NQ],
                        start=(dc == 0), stop=(dc == 3))

            # ---- harmonic bases + score matmuls, interleaved per k so
            # every engine pipelines: ACT computes the base sin/cos pair and
            # the squares S_m^2/C_1^2 that feed the even-cosine and cubic
            # identities; the DVE runs the keys-side recurrences + w2b mults;
            # GPSIMD runs the (8x smaller) query-side chains; the PE consumes
            # each harmonic's tiles as they appear. ----
            SK, CK, SQ, CQ = {}, {}, {}, {}
            HW2 = 2 * SL          # half width: kp-side ops run in 2 halves
            HSL = [slice(0, HW2), slice(HW2, 2 * HW2)]
            SK[1] = cp.tile([128, 4 * SL], F16, tag="kS1", name="kS1")
            CK[1] = cp.tile([128, 4 * SL], F16, tag="kC1", name="kC1")
            sq1 = cp.tile([128, 4 * SL], F16, tag="ksq1", name="ksq1")
            two_c1k = cp.tile([128, 4 * SL], F16, tag="k2C1", name="k2C1")
            for hs in HSL:
                nc.scalar.activation(SK[1][:, hs], kps[:, hs], AF.Sin,
                                     scale=OMEGA1)
                nc.scalar.activation(CK[1][:, hs], kps[:, hs], AF.Sin,
                                     bias=halfpi[:, 0:1], scale=OMEGA1)
                nc.scalar.square(sq1[:, hs], SK[1][:, hs])
                nc.vector.tensor_scalar_mul(two_c1k[:, hs], CK[1][:, hs], 2.0)
            SQ[1] = cp.tile([128, 4 * NQ], F16, tag="qS1", name="qS1")
            nc.scalar.activation(SQ[1][:], qps[:], AF.Sin, scale=OMEGA1)
            CQ[1] = cp.tile([128, 4 * NQ], F16, tag="qC1", name="qC1")
            nc.scalar.activation(CQ[1][:], qps[:], AF.Sin,
                                 bias=halfpi[:, 0:1], scale=OMEGA1)
            two_c1q = cp.tile([128, 4 * NQ], F16, tag="q2C1", name="q2C1")
            nc.gpsimd.tensor_add(two_c1q[:], CQ[1][:], CQ[1][:])
            ones_q = cp.tile([128, 4 * NQ], F16, tag="qones", name="qones")
            nc.gpsimd.memset(ones_q[:], 1.0)

            def ktile(nm):
                return cp.tile([128, 4 * SL], F16, tag=nm, name=nm)

            t3 = ktile("kt3")
            SQS = {}
            SIGMA = {k: (2 * SIGMA_BASE(k // 2) if k % 2 == 0 else 1)
                     for k in range(1, K_HARM + 1)}
            c1muls = {2: two_c1k}

            def c1mul(f):
                if f not in c1muls:
                    t = ktile(f"kC1x{f}")
                    for hs2 in HSL:
                        nc.vector.tensor_scalar_mul(t[:, hs2],
                                                    CK[1][:, hs2], float(f))
                    c1muls[f] = t
                return c1muls[f]

            def k_step(k, hs):
                """Keys-side S_k, C_k on one half-slice (DVE + ACT squares):
                k=2: S2 = 2C1*S1,          C2 = 1 - 2*sq1 (in place)
                k=3: S3 = S1*(2*C2 + 1),   C3 = 2C1*C2 - C1
                even k=2m>2: S = (2*S_m)*C_m fused, C = 1 - 2*Sm^2 (ACT sq)
                other odd:   step recurrence 2C1*X_{k-1} - X_{k-2}
                """
                if k == 2:
                    nc.vector.tensor_mul(SK[2][:, hs], SK[1][:, hs],
                                         CK[1][:, hs])
                    nc.vector.tensor_scalar(sq1[:, hs], sq1[:, hs], -2.0, 1.0,
                                            ALU.mult, ALU.add)
                elif k == 3:
                    nc.vector.tensor_scalar(t3[:, hs], CK[2][:, hs], 2.0, 1.0,
                                            ALU.mult, ALU.add)
                    nc.vector.tensor_mul(SK[3][:, hs], SK[1][:, hs],
                                         t3[:, hs])
                    nc.vector.tensor_mul(CK[3][:, hs], two_c1k[:, hs],
                                         CK[2][:, hs])
                    nc.vector.tensor_sub(CK[3][:, hs], CK[3][:, hs],
                                         CK[1][:, hs])
                elif k % 2 == 0:
                    # stored S~_k = S_k/sigma_k, so the doubling is a plain
                    # 2x-rate tensor_mul; sigma is absorbed into w2bc and
                    # the square's TS constant
                    m = k // 2
                    nc.vector.tensor_mul(SK[k][:, hs], SK[m][:, hs],
                                         CK[m][:, hs])
                    nc.vector.tensor_scalar(CK[k][:, hs], SQS[m][:, hs],
                                            -2.0 * SIGMA[m] ** 2, 1.0,
                                            ALU.mult, ALU.add)
                else:
                    nc.vector.tensor_mul(SK[k][:, hs], c1mul(2 * SIGMA[k - 1])[:, hs],
                                         SK[k - 1][:, hs])
                    nc.vector.tensor_sub(SK[k][:, hs], SK[k][:, hs],
                                         SK[k - 2][:, hs])
                    nc.vector.tensor_mul(CK[k][:, hs], two_c1k[:, hs],
                                         CK[k - 1][:, hs])
                    nc.vector.tensor_sub(CK[k][:, hs], CK[k][:, hs],
                                         CK[k - 2][:, hs])

            # scores = sum_k (w2 b_k sin_k(qp))^T cos_k(kp)
            #        + (w2 b_k cos_k(qp))^T sin_k(kp)  in one PSUM bank
            sc = sp.tile([NQ, SL], F32, tag="sc", name="sc")
            n_mm = K_HARM * 2 * 4
            i_mm = 0
            for k in range(1, K_HARM + 1):
                if k >= 2:
                    SK[k] = ktile(f"kS{k}")
                    CK[k] = sq1 if k == 2 else ktile(f"kC{k}")
                    for hs in HSL:
                        k_step(k, hs)
                    if 2 * k <= K_HARM:
                        # ACT square of S_k now -> feeds C_{2k} later
                        SQS[k] = ktile(f"ksqs{k}")
                        for hs in HSL:
                            nc.scalar.square(SQS[k][:, hs], SK[k][:, hs])
                    _chain_step(nc, nc.gpsimd, cp, SQ, CQ, two_c1q, ones_q,
                                k, 4 * NQ, "q")
                sqw = cp.tile([128, 4 * NQ], F16, tag=f"sqw{k}", name=f"sqw{k}")
                nc.vector.tensor_mul(
                    sqw[:], SQ[k][:], w2b[:, (k - 1) * 4 * NQ:k * 4 * NQ])
                cqw = cp.tile([128, 4 * NQ], F16, tag=f"cqw{k}", name=f"cqw{k}")
                nc.vector.tensor_mul(
                    cqw[:], CQ[k][:], w2bc[:, (k - 1) * 4 * NQ:k * 4 * NQ])
                for qside, kside in ((cqw, SK[k]), (sqw, CK[k])):
                    for hc in range(4):
                        nc.tensor.matmul(
                            sc[:],
                            qside[:, hc * NQ:(hc + 1) * NQ],
                            kside[:, hc * SL:(hc + 1) * SL],
                            start=(i_mm == 0), stop=(i_mm == n_mm - 1))
                        i_mm += 1

            # ---- softmax numerator/denominator (no max-subtract needed:
            #      |scores| <= sum|w2| ~ 11.5, exp stays in fp32 range) ----
            esc = cp.tile([NQ, SL], F16, tag="esc", name="esc")
            den = cp.tile([NQ, 1], F32, tag="den", name="den")
            nc.scalar.activation(esc[:], sc[:], AF.Exp, accum_out=den[:])

            # ---- out_u = exp(scores) @ values  (transpose exp via PE) ----
            av = sp.tile([NQ, V], F32, tag="av", name="av")
            for i in range(4):
                pt = tp.tile([128, NQ], F16, tag="pt", name="pt")
                nc.tensor.transpose(pt[:], esc[:, i * 128:(i + 1) * 128], eye[:])
                et = cp.tile([128, NQ], F16, tag=f"et{i}", name=f"et{i}")
                nc.vector.tensor_copy(et[:], pt[:])
                nc.tensor.matmul(av[:], et[:], vals[:, i * V:(i + 1) * V],
                                 start=(i == 0), stop=(i == 3))
            outu = cp.tile([NQ, V], F32, tag="outu", name="outu")
            nc.scalar.copy(outu[:], av[:])
            nc.sync.dma_start(d_outu[:], outu[:])
            nc.sync.dma_start(d_den[:], den[:])

    nc.compile()
    return nc


def _prep_inputs(queries, keys, values, W1, b1, w2):
    """Per-core input dicts.  Core c = batch (c//2), s-half (c%2)."""
    W1q = np.ascontiguousarray(W1[:, :D])
    W1k = np.ascontiguousarray(W1[:, D:])
    # lhsT blocks [d, h], hc-major: free index = (hc*4+dc)*128 + h_in so the
    # projection for output chunk hc only needs one contiguous DMA chunk
    w1qT = W1q.T.reshape(4, 128, 4, 128).transpose(1, 2, 0, 3).reshape(128, -1)
    w1kT = W1k.T.reshape(4, 128, 4, 128).transpose(1, 2, 0, 3).reshape(128, -1)
    b1r = b1.reshape(1, 512).astype(np.float16)
    ones_r = np.ones((1, 512), dtype=np.float16)
    # w2b[p, ((k*4)+hc)*NQ + q] = w2[hc*128+p] * BCOEF[k]  (replicated over q)
    w2hc = w2.reshape(4, 128).T                          # [128, 4]

    def _coef(scales):
        c = (w2hc[:, None, :, None] * (_BCOEF * scales)[None, :, None, None]
             ).astype(np.float16).reshape(128, K_HARM * 4, 1)
        c = np.broadcast_to(c, (128, K_HARM * 4, NQ)).reshape(128, -1)
        return np.ascontiguousarray(c)

    sig = np.array([1 if k % 2 else 2 * SIGMA_BASE(k // 2)
                    for k in range(1, K_HARM + 1)], dtype=np.float64)
    w2b = _coef(np.ones(K_HARM))
    w2bc = _coef(sig)
    eye = np.eye(NQ, dtype=np.float16)

    in_maps = []
    for c in range(NCORES):
        bi, sh = c // 2, c % 2
        qT = queries[bi].T.reshape(4, 128, NQ).transpose(1, 0, 2).reshape(128, -1)
        keysT = (keys[bi, sh * SL:(sh + 1) * SL].T
                 .reshape(4, 128, SL).transpose(1, 0, 2).reshape(128, -1))
        vals = (values[bi, sh * SL:(sh + 1) * SL]
                .reshape(4, 128, V).transpose(1, 0, 2).reshape(128, -1))
        in_maps.append({
            "qT": np.ascontiguousarray(qT, dtype=np.float16),
            "keysT": np.ascontiguousarray(keysT, dtype=np.float16),
            "vals": np.ascontiguousarray(vals, dtype=np.float16),
            "w1qT": np.ascontiguousarray(w1qT, dtype=np.float16),
            "w1kT": np.ascontiguousarray(w1kT, dtype=np.float16),
            "b1r": b1r,
            "ones_r": ones_r,
            "w2b": w2b,
            "w2bc": w2bc,
            "eye64": eye,
        })
    return in_maps


def run(inputs_by_core, trace=False, **kw):
    if "nc" not in _PROG_CACHE:
        _PROG_CACHE["nc"] = _build_program()
    return run_bass_kernel_spmd(_PROG_CACHE["nc"], inputs_by_core,
                                core_ids=list(range(NCORES)), trace=trace, **kw)


def kernel(queries, keys, values, W1, b1, w2):
    queries = np.asarray(queries, dtype=np.float32)
    keys = np.asarray(keys, dtype=np.float32)
    values = np.asarray(values, dtype=np.float32)
    W1 = np.asarray(W1, dtype=np.float32)
    b1 = np.asarray(b1, dtype=np.float32)
    w2 = np.asarray(w2, dtype=np.float32)

    res = run(_prep_inputs(queries, keys, values, W1, b1, w2))
    out = np.empty((B, NQ, V), dtype=np.float32)
    for bi in range(B):
        u0, d0 = res.results[2 * bi]["out_u"], res.results[2 * bi]["den"]
        u1, d1 = res.results[2 * bi + 1]["out_u"], res.results[2 * bi + 1]["den"]
        out[bi] = (u0 + u1) / (d0 + d1)
    return out

